# revision 22
# baseline (speedup 1.0000x reference)
"""GCN+MLP Trainium2 kernel: single Bass/Tile NEFF over 8 cores.

Model (reference): h0 = x@We + be; two ChebConv(K=2) layers
  h_{l+1} = relu(h_l @ W0 + (S @ h_l) @ W1 + b),  S = scatter(wgt), wgt =
  -dinv[src]*dinv[dst]; then per-batch mean pool -> 2-layer MLP -> broadcast.

Distribution: dst-sharded, 8192 nodes (4 batches) per core.  Edges are routed
on host to the dst owner, sorted by (src_half, dst_group) and chunked into
128-edge matmul chunks.  Per chunk the kernel builds a weighted one-hot
oh[slot, dst_local] = (iota==dst)*w on the vector engine (or, for a fraction
of chunks, on the scalar engine as w*relu(1-(iota-dst)^2)) and accumulates
  psum[feat, dst] += gathered_rows[slot, feat]^T @ oh
on the tensor engine.  Weights are sign-flipped on host (w = +dinv*dinv,
W1 -> -W1) so the scalar-engine path stays in relu range.  Layer-1 "gather"
is done on host (x rows in edge order, streamed sequentially).  Layer-2
gathers rows of u2 = h1 @ (-W11) from an all-gathered HBM table with the Q7
dma_gather extended instruction in flat blocks of 8 chunks (1024 int16
indices, the HW cap) spread over 4 SWDGE queues; edges are pre-split by
src < 32768 so each gather uses a single table base.  Algebraic folds:
W0e = We@W00 and Wf = We@(-W01) fold the embed matmul away; u2 folds the
conv1 W1 matmul into the gather table (computed node-major directly as
h1T_g^T @ W11n, no transpose pass); the embed bias enters as a rank-1
(v0 x sw) matmul and a constant bias vector.
"""

import sys

sys.path.insert(0, "/opt/trn_rl_repo")

import numpy as np
import ml_dtypes

B, E, D = 32, 2048, 64
EMB, HID, PRED, NPRED = 128, 64, 3, 12
N = B * E
NCORES = 8
NPC = N // NCORES          # 8192 nodes/core
NGRP = NPC // 128          # 64 dst groups of 128 per core
HALF = N // 2              # int16 index limit -> split table in two halves
BLK = 8                    # chunks per dma_gather (1024 idx = HW cap)
ACT_FRAC = 5               # of every 16 one-hots, this many go to ScalarE

_CACHE = {}
bf = ml_dtypes.bfloat16


def _prep(src, dst, wgt_pos, x):
    """Route edges to (core, src-half, group), chunk into 128-edge chunks.

    Chunk axis is class-major: all class-0 (src < HALF) chunks for groups
    0..63 first, then all class-1 chunks.  Chunk counts per (group, half) are
    maxed over cores so all cores share one program.
    """
    core = dst >> 13
    grp = (dst & (NPC - 1)) >> 7
    half = (src >= HALF).astype(np.int64)
    key = ((core * 2 + half) * NGRP + grp)
    order = np.argsort(key, kind="stable")
    src_s, dst_s, wgt_s = src[order], dst[order], wgt_pos[order]
    bounds = np.searchsorted(key[order], np.arange(NCORES * NGRP * 2 + 1))
    cnt = np.diff(bounds).reshape(NCORES, 2, NGRP)
    nch = np.ceil(cnt.max(axis=0) / 128).astype(np.int64).T   # [NGRP, 2]
    t0 = int(nch[:, 0].sum())
    ncht = int(nch.sum())
    ch_start = np.zeros((NGRP, 2), np.int64)
    ch_start[:, 0] = np.cumsum(nch[:, 0]) - nch[:, 0]
    ch_start[:, 1] = t0 + np.cumsum(nch[:, 1]) - nch[:, 1]

    xg = np.zeros((NCORES, 128, ncht, D), bf)
    dst_t = np.zeros((NCORES, 128, ncht), np.float32)
    dst_n = np.zeros((NCORES, 128, ncht), np.float32)
    wgt_t = np.zeros((NCORES, 128, ncht), np.float32)
    wgt_n = np.zeros((NCORES, 128, ncht), np.float32)
    idx_t = np.zeros((NCORES, 128, ncht * 8), np.int16)
    for c in range(NCORES):
        for h in range(2):
            for g in range(NGRP):
                lo = bounds[(c * 2 + h) * NGRP + g]
                hi = bounds[(c * 2 + h) * NGRP + g + 1]
                n = hi - lo
                if n == 0:
                    continue
                s_e = src_s[lo:hi]
                d_e = (dst_s[lo:hi] & 127).astype(np.float32)
                w_e = wgt_s[lo:hi]
                q0 = ch_start[g, h]
                sl = np.arange(n)
                ch, pt = q0 + sl // 128, sl % 128
                xg[c, pt, ch] = x[s_e]
                dst_t[c, pt, ch] = d_e
                dst_n[c, pt, ch] = -d_e
                wgt_t[c, pt, ch] = w_e
                wgt_n[c, pt, ch] = -w_e.astype(bf).astype(np.float32)
                flat = q0 * 128 + sl
                fcol, prow = flat // 16, flat % 16
                v = (s_e - h * HALF).astype(np.int16)
                for rep in range(8):
                    idx_t[c, prow + rep * 16, fcol] = v
    return nch, ch_start, t0, ncht, xg, dst_t, dst_n, wgt_t, wgt_n, idx_t


def _host(inputs):
    x = np.asarray(inputs["x"], np.float32).reshape(N, D)
    edge_index = np.asarray(inputs["edge_index"])
    We = np.asarray(inputs["embed_W"], np.float32)
    be = np.asarray(inputs["embed_b"], np.float32)
    W00 = np.asarray(inputs["conv0_W0"], np.float32)
    W01 = np.asarray(inputs["conv0_W1"], np.float32)
    b0 = np.asarray(inputs["conv0_b"], np.float32)
    W10 = np.asarray(inputs["conv1_W0"], np.float32)
    W11 = np.asarray(inputs["conv1_W1"], np.float32)
    b1 = np.asarray(inputs["conv1_b"], np.float32)
    mW1 = np.asarray(inputs["mlp_W1"], np.float32)
    mb1 = np.asarray(inputs["mlp_b1"], np.float32)
    mW2 = np.asarray(inputs["mlp_W2"], np.float32)
    mb2 = np.asarray(inputs["mlp_b2"], np.float32)

    src = np.asarray(edge_index[0]).astype(np.int64)
    dst = np.asarray(edge_index[1]).astype(np.int64)
    deg = np.bincount(src, minlength=N).astype(np.float32)
    dinv = np.where(deg > 0, 1.0 / np.sqrt(np.maximum(deg, 1e-12)), 0.0)
    wgt_pos = (dinv[src] * dinv[dst]).astype(np.float32)   # = -true wgt

    nch, ch_start, t0, ncht, xg, dst_t, dst_n, wgt_t, wgt_n, idx_t = _prep(
        src, dst, wgt_pos, x)

    sw_pos = np.zeros(N, np.float32)
    np.add.at(sw_pos, dst, wgt_pos)

    xT_sh = x.reshape(NCORES, NPC, D).transpose(0, 2, 1).astype(bf)
    consts = {
        "W0e": (We @ W00).astype(bf),
        "Wf": (-(We @ W01)).astype(bf),
        "v0": (-(W01.T @ be)).reshape(1, EMB).astype(bf),
        "W10": W10.astype(bf),
        "W11n": (-W11).astype(bf),
        "bb0": (b0 + W00.T @ be).reshape(EMB, 1).astype(np.float32),
        "b1": b1.reshape(EMB, 1).astype(np.float32),
        "mW1": (mW1 / E).astype(bf),
        "mb1": mb1.reshape(HID, 1).astype(np.float32),
        "mW2": mW2.astype(bf),
        "mb2": mb2.reshape(PRED, 1).astype(np.float32),
        "iota": np.tile(np.arange(128, dtype=np.float32), (128, 1)).astype(bf),
    }
    in_maps = []
    for c in range(NCORES):
        m = {
            "xT": np.ascontiguousarray(xT_sh[c]),
            "xg": np.ascontiguousarray(xg[c]),
            "dstc": np.ascontiguousarray(dst_t[c]),
            "dstn": np.ascontiguousarray(dst_n[c]),
            "wgtc": np.ascontiguousarray(wgt_t[c]),
            "wgtn": np.ascontiguousarray(wgt_n[c]),
            "idxc": np.ascontiguousarray(idx_t[c]),
            "sw": np.ascontiguousarray(
                sw_pos.reshape(NCORES, 1, NPC)[c].astype(bf)),
        }
        m.update(consts)
        in_maps.append(m)
    return nch, ch_start, t0, ncht, in_maps


def _build_nc(nch, ch_start, t0, ncht, n_cores, phase=3):
    import concourse.bacc as bacc
    import concourse.bass as bass
    import concourse.mybir as mybir
    import concourse.tile as tile

    f32 = mybir.dt.float32
    bf16 = mybir.dt.bfloat16
    i16 = mybir.dt.int16
    AF = mybir.ActivationFunctionType
    OP = mybir.AluOpType

    nc = bacc.Bacc("TRN2", target_bir_lowering=False, num_devices=n_cores,
                   num_swdge_queues=4)

    t_xT = nc.dram_tensor("xT", [D, NPC], bf16, kind="ExternalInput")
    t_xg = nc.dram_tensor("xg", [128, ncht, D], bf16, kind="ExternalInput")
    t_dst = nc.dram_tensor("dstc", [128, ncht], f32, kind="ExternalInput")
    t_dstn = nc.dram_tensor("dstn", [128, ncht], f32, kind="ExternalInput")
    t_wgt = nc.dram_tensor("wgtc", [128, ncht], f32, kind="ExternalInput")
    t_wgtn = nc.dram_tensor("wgtn", [128, ncht], f32, kind="ExternalInput")
    t_idx = nc.dram_tensor("idxc", [128, ncht * 8], i16, kind="ExternalInput")
    t_sw = nc.dram_tensor("sw", [1, NPC], bf16, kind="ExternalInput")
    t_W0e = nc.dram_tensor("W0e", [D, EMB], bf16, kind="ExternalInput")
    t_Wf = nc.dram_tensor("Wf", [D, EMB], bf16, kind="ExternalInput")
    t_v0 = nc.dram_tensor("v0", [1, EMB], bf16, kind="ExternalInput")
    t_W10 = nc.dram_tensor("W10", [EMB, EMB], bf16, kind="ExternalInput")
    t_W11 = nc.dram_tensor("W11n", [EMB, EMB], bf16, kind="ExternalInput")
    t_bb0 = nc.dram_tensor("bb0", [EMB, 1], f32, kind="ExternalInput")
    t_b1 = nc.dram_tensor("b1", [EMB, 1], f32, kind="ExternalInput")
    t_mW1 = nc.dram_tensor("mW1", [EMB, HID], bf16, kind="ExternalInput")
    t_mb1 = nc.dram_tensor("mb1", [HID, 1], f32, kind="ExternalInput")
    t_mW2 = nc.dram_tensor("mW2", [HID, PRED], bf16, kind="ExternalInput")
    t_mb2 = nc.dram_tensor("mb2", [PRED, 1], f32, kind="ExternalInput")
    t_iota = nc.dram_tensor("iota", [128, 128], bf16, kind="ExternalInput")
    t_out = nc.dram_tensor("o", [PRED, B // n_cores], f32,
                           kind="ExternalOutput")

    t_ush = nc.dram_tensor("ushard", [NPC, EMB], bf16, kind="Internal")
    t_ufull = nc.dram_tensor("ufull", [n_cores * NPC, EMB], bf16,
                             kind="Internal", addr_space="Shared")

    BPC = B // n_cores
    nblk = [(t0 + BLK - 1) // BLK, (ncht - t0 + BLK - 1) // BLK]
    base = [0, t0]

    with tile.TileContext(nc) as tc:
        with (
            tc.tile_pool(name="const", bufs=1) as cpool,
            tc.tile_pool(name="xg", bufs=4) as xgpool,
            tc.tile_pool(name="oh", bufs=8) as ohpool,
            tc.tile_pool(name="gat", bufs=8) as gatpool,
            tc.tile_pool(name="ev", bufs=4) as evpool,
            tc.tile_pool(name="psA", bufs=3, space=bass.MemorySpace.PSUM) as psApool,
            tc.tile_pool(name="ps1", bufs=2, space=bass.MemorySpace.PSUM) as ps1pool,
            tc.tile_pool(name="psU", bufs=2, space=bass.MemorySpace.PSUM) as psUpool,
            tc.tile_pool(name="psM", bufs=1, space=bass.MemorySpace.PSUM) as psMpool,
        ):
            xT = cpool.tile([D, NPC], bf16)
            dstc = cpool.tile([128, ncht], f32)
            dstn = cpool.tile([128, ncht], f32)
            wgtc = cpool.tile([128, ncht], f32)
            wgtn = cpool.tile([128, ncht], f32)
            idxc = cpool.tile([128, ncht * 8], i16)
            sw = cpool.tile([1, NPC], bf16)
            W0e = cpool.tile([D, EMB], bf16)
            Wf = cpool.tile([D, EMB], bf16)
            v0 = cpool.tile([1, EMB], bf16)
            W10 = cpool.tile([EMB, EMB], bf16)
            W11n = cpool.tile([EMB, EMB], bf16)
            bb0 = cpool.tile([EMB, 1], f32)
            b1 = cpool.tile([EMB, 1], f32)
            mW1 = cpool.tile([EMB, HID], bf16)
            mb1 = cpool.tile([HID, 1], f32)
            mW2 = cpool.tile([HID, PRED], bf16)
            mb2 = cpool.tile([PRED, 1], f32)
            iota = cpool.tile([128, 128], bf16)
            h1T = cpool.tile([EMB, NPC], bf16)
            h2T = cpool.tile([EMB, NPC], bf16)

            for t_src_, t_dst_ in [
                (t_xT, xT), (t_dst, dstc), (t_dstn, dstn), (t_wgt, wgtc),
                (t_wgtn, wgtn), (t_idx, idxc), (t_sw, sw), (t_W0e, W0e),
                (t_Wf, Wf), (t_v0, v0), (t_W10, W10), (t_W11, W11n),
                (t_bb0, bb0), (t_b1, b1), (t_mW1, mW1), (t_mb1, mb1),
                (t_mW2, mW2), (t_mb2, mb2), (t_iota, iota),
            ]:
                nc.sync.dma_start(t_dst_[:], t_src_[:])

            ohctr = [0]

            def build_oh(q):
                oh = ohpool.tile([128, 128], bf16, tag="oh")
                k = ohctr[0] % 16
                ohctr[0] += 1
                if k < ACT_FRAC:
                    s_t = ohpool.tile([128, 128], bf16, tag="sq")
                    nc.scalar.activation(s_t[:], iota[:], AF.Square,
                                         bias=dstn[:, q:q + 1])
                    nc.scalar.activation(oh[:], s_t[:], AF.Relu,
                                         bias=wgtc[:, q:q + 1],
                                         scale=wgtn[:, q:q + 1])
                else:
                    nc.vector.tensor_scalar(
                        oh[:], iota[:], dstc[:, q:q + 1], wgtc[:, q:q + 1],
                        OP.is_equal, OP.mult)
                return oh

            # ---------------- layer 1 ----------------
            for g in range(NGRP):
                gsl = slice(g * 128, (g + 1) * 128)
                runs = [(int(ch_start[g, h]), int(nch[g, h])) for h in (0, 1)]
                nchg = runs[0][1] + runs[1][1]
                psA = psApool.tile([D, 128], f32)
                i = 0
                for q0, nh in runs:
                    if nh == 0:
                        continue
                    xg_t = xgpool.tile([128, nh, D], bf16, tag="xg")
                    nc.sync.dma_start(xg_t[:], t_xg[:, q0:q0 + nh, :])
                    for j in range(nh):
                        oh = build_oh(q0 + j)
                        nc.tensor.matmul(psA[:], xg_t[:, j, :], oh[:],
                                         start=(i == 0), stop=(i == nchg - 1))
                        i += 1
                A_s = evpool.tile([D, 128], bf16, tag="As")
                nc.vector.tensor_copy(A_s[:], psA[:])
                ps1 = ps1pool.tile([EMB, 128], f32)
                nc.tensor.matmul(ps1[:], W0e[:], xT[:, gsl],
                                 start=True, stop=False)
                nc.tensor.matmul(ps1[:], v0[:], sw[:, gsl],
                                 start=False, stop=False)
                nc.tensor.matmul(ps1[:], Wf[:], A_s[:], start=False, stop=True)
                nc.scalar.activation(h1T[:, gsl], ps1[:], AF.Relu, bias=bb0[:])
                # u2 shard, node-major: u2_g = h1T_g^T @ W11n
                psU = psUpool.tile([128, EMB], f32)
                nc.tensor.matmul(psU[:], h1T[:, gsl], W11n[:],
                                 start=True, stop=True)
                u_s = evpool.tile([128, EMB], bf16, tag="us")
                nc.scalar.activation(u_s[:], psU[:], AF.Copy)
                nc.sync.dma_start(t_ush[gsl, :], u_s[:])

            # ---------------- all-gather ----------------
            if phase >= 2:
                nc.gpsimd.collective_compute(
                    "AllGather", OP.bypass,
                    replica_groups=[list(range(n_cores))],
                    ins=[t_ush.ap()], outs=[t_ufull.ap()])

            # ---------------- layer 2 ----------------
            qrr = [0]
            blocks = [{}, {}]

            def get_block(h, b):
                if b not in blocks[h]:
                    nchb = min(BLK, (ncht if h else t0) - base[h] - b * BLK)
                    gt = gatpool.tile([128, BLK, EMB], bf16, tag="gt")
                    if phase >= 3:
                        q0 = base[h] + b * BLK
                        nidx = nchb * 128
                        nc.gpsimd.dma_gather(
                            gt[:, :nchb, :],
                            t_ufull[h * HALF:(h + 1) * HALF, :],
                            idxc[:, q0 * 8:q0 * 8 + nidx // 16],
                            nidx, nidx, EMB, queue_num=qrr[0] % 4)
                        qrr[0] += 1
                    else:
                        nc.vector.memset(gt[:], 0.0)
                    blocks[h][b] = gt
                return blocks[h][b]

            for g in range(NGRP):
                gsl = slice(g * 128, (g + 1) * 128)
                runs = [(int(ch_start[g, h]), int(nch[g, h])) for h in (0, 1)]
                nchg = runs[0][1] + runs[1][1]
                ps1 = ps1pool.tile([EMB, 128], f32)
                nc.tensor.matmul(ps1[:], W10[:], h1T[:, gsl],
                                 start=True, stop=False)
                i = 0
                for h, (q0, nh) in enumerate(runs):
                    for j in range(nh):
                        q = q0 + j
                        rel = q - base[h]
                        gt = get_block(h, rel // BLK)
                        oh = build_oh(q)
                        nc.tensor.matmul(ps1[:], gt[:, rel % BLK, :], oh[:],
                                         start=False, stop=(i == nchg - 1))
                        i += 1
                nc.scalar.activation(h2T[:, gsl], ps1[:], AF.Relu, bias=b1[:])

            # ---------------- pool + MLP ----------------
            hm_f = evpool.tile([EMB, BPC], f32, tag="hmf")
            nc.vector.tensor_reduce(
                out=hm_f[:],
                in_=h2T[:].rearrange("p (b e) -> p b e", b=BPC),
                op=OP.add, axis=mybir.AxisListType.X)
            hm = evpool.tile([EMB, BPC], bf16, tag="hm")
            nc.vector.tensor_copy(hm[:], hm_f[:])
            psM = psMpool.tile([HID, BPC], f32)
            nc.tensor.matmul(psM[:], mW1[:], hm[:], start=True, stop=True)
            z = evpool.tile([HID, BPC], bf16, tag="z")
            nc.scalar.activation(z[:], psM[:], AF.Relu, bias=mb1[:])
            psO = psMpool.tile([PRED, BPC], f32, tag="psM")
            nc.tensor.matmul(psO[:], mW2[:], z[:], start=True, stop=True)
            o_s = evpool.tile([PRED, BPC], f32, tag="os")
            nc.vector.tensor_scalar(o_s[:], psO[:], mb2[:], None, OP.add)
            nc.sync.dma_start(t_out[:], o_s[:])

    nc.compile()
    return nc


def kernel(**inputs):
    nch, ch_start, t0, ncht, in_maps = _host(inputs)

    key = (tuple(nch.ravel()), ncht)
    if _CACHE.get("key") != key:
        _CACHE["nc"] = _build_nc(nch, ch_start, t0, ncht, NCORES)
        _CACHE["key"] = key
    nc = _CACHE["nc"]

    from concourse.bass_utils import run_bass_kernel_spmd

    res = run_bass_kernel_spmd(nc, in_maps, list(range(NCORES)))
    out1 = np.zeros((B, PRED), np.float32)
    for c in range(NCORES):
        o = np.asarray(res.results[c]["o"], np.float32)   # [PRED, BPC]
        out1[c * (B // NCORES):(c + 1) * (B // NCORES), :] = o.T
    full = np.broadcast_to(out1[:, None, None, :], (B, NPRED, E, PRED))
    return np.ascontiguousarray(full, dtype=np.float32)


kernel._jit_holder = _CACHE
